# revision 25
# baseline (speedup 1.0000x reference)
"""Multi-head causal self-attention (B=2, S=2048, D=1024, H=16) on 8 TRN2
NeuronCores via Bass/Tile.

Sharding: core c -> (batch b = c // 4, head-group g = c % 4). Each core
computes q/k/v projections for its 4 heads (256 of 1024 projection cols),
causal flash attention for those heads, and a partial output projection
(row-parallel over the head dim). Host sums the 4 partials per batch.

Device layouts (all transposed so the contraction dim sits on partitions):
  xT   [D, S]   : x[b].T, host-transposed
  Q^T/K^T [e, S]: head dim on partitions
  V    [k, e+1] : natural, with a ones column per head; the ones column turns
                  the AV^T matmul into (unnormalized AV^T, softmax denom) rows
  A^T  [e, S]   : produced directly by AV^T matmul, consumed as moving
                  operand of the output projection -> zero on-chip transposes
  outT [D, S]   : transposed partial output, host sums + transposes back

All matmul operands are bf16 (f32r streams at ~2 cycles/col on the PE and
disables fast-weight-load; bf16 streams 1 col/cycle, halves DMA, and doubles
DVE throughput). PSUM accumulation stays fp32; rel err ~1e-2 budget vs the
2e-2 gate. Softmax: scores S^T[k, q] so the partition-dim softmax reduction
folds into the AV matmul via the ones column; exp needs no max-subtraction
(scores are O(1)). The two heads of an e-block are interleaved per k-block:
their K=64 score matmuls sit in disjoint PE row-groups (tile_position (0,0)
vs (64,0)) and overlap in the array. The softmax reciprocal uses the ~5x
faster reciprocal_approx_fast, broadcast over 64 partitions as a K=1 outer
product (f32-bitcast-to-f32r operands).
"""

from contextlib import ExitStack

import numpy as np

import concourse.bass as bass
import concourse.mybir as mybir
import concourse.tile as tile
from concourse.bass_utils import run_bass_kernel_spmd

# Problem constants (hardcoded per harness contract).
B, S, D, NH, DH = 2, 2048, 1024, 16, 64
N_CORES = 8
GROUPS = 4                 # head-groups; cores per batch
HPC = NH // GROUPS         # heads per core = 4
E = HPC * DH               # per-core projection width = 256
P = 128                    # SBUF partitions
SC = 512                   # moving-operand chunk (q chunk)
ND = D // P                # 8 d-chunks
NEB = E // P               # 2 e-blocks per core
NQ = S // SC               # 4 q chunks
NKB = S // P               # 16 k blocks
SCALE = DH ** -0.5

F32 = mybir.dt.float32
F32R = mybir.dt.float32r
MM_DT = mybir.dt.bfloat16


def _split_multiwait(nc, max_waits=1):
    """This toolchain's walrus codegen accepts at most one sync-wait per
    instruction ("Too many sync wait commands"). Tile emits multi-wait
    instructions (notably the kernel-tail Drain). Keep the last wait (+ all
    updates) on the original instruction and hoist earlier waits onto
    single-wait Drains inserted before it on the same engine."""
    for f in nc.m.functions:
        for bb in f.blocks:
            new = []
            changed = False
            for inst in bb.instructions:
                si = inst.sync_info
                waits = list(si.on_wait) if si is not None and si.on_wait else []
                if len(waits) > max_waits:
                    for j, w in enumerate(waits[:-max_waits]):
                        d = mybir.InstDrain(name=f"{inst.name}-sw{j}", ins=[], outs=[])
                        d.engine = inst.engine
                        d.sync_info = mybir.SyncInfo(on_wait=[w], on_update=[])
                        new.append(d)
                    inst.sync_info = mybir.SyncInfo(
                        on_wait=waits[-max_waits:],
                        on_update=list(si.on_update) if si.on_update else [],
                    )
                    changed = True
                new.append(inst)
            if changed:
                bb.instructions = new


def build_nc(repeat=1, pav_bufs=4, psc_bufs=3, pmx_bufs=1, ptp_bufs=8):
    """repeat>1 wraps the whole body in a hardware For_i loop — used only by
    the benchmark to amortize dispatch overhead out of wall-clock timing."""
    nc = bass.Bass("TRN2", target_bir_lowering=False, debug=False,
                   num_devices=N_CORES)

    xT = nc.dram_tensor("xT", [D, S], MM_DT, kind="ExternalInput")
    wqT = nc.dram_tensor("wqT", [D, E], MM_DT, kind="ExternalInput")
    wkT = nc.dram_tensor("wkT", [D, E], MM_DT, kind="ExternalInput")
    wvT = nc.dram_tensor("wvT", [D, E], MM_DT, kind="ExternalInput")
    woT = nc.dram_tensor("woT", [E, D], MM_DT, kind="ExternalInput")
    bq = nc.dram_tensor("bq", [E], F32, kind="ExternalInput")
    bk = nc.dram_tensor("bk", [E], F32, kind="ExternalInput")
    outT = nc.dram_tensor("outT", [D, S], F32, kind="ExternalOutput")

    AF = mybir.ActivationFunctionType
    with tile.TileContext(nc) as tc:
        with ExitStack() as ctx:
            if repeat > 1:
                ctx.enter_context(tc.For_i(0, repeat, 1))
            const = ctx.enter_context(tc.tile_pool(name="const", bufs=1))

            # ---- persistent SBUF tensors (chunked for fine-grained deps) ----
            x_sbs = [const.tile([P, S], MM_DT, tag=f"x{i}", name=f"x{i}") for i in range(ND)]
            # q/k/v weights as single tiles (one coalesced DMA each); the
            # per-d-block views keep the matmul call sites unchanged
            wq_sb = const.tile([P, ND, E], MM_DT, tag="wq", name="wq")
            wk_sb = const.tile([P, ND, E], MM_DT, tag="wk", name="wk")
            wv_sb = const.tile([P, ND, E], MM_DT, tag="wv", name="wv")
            wq_sbs = [wq_sb[:, i, :] for i in range(ND)]
            wk_sbs = [wk_sb[:, i, :] for i in range(ND)]
            wv_sbs = [wv_sb[:, i, :] for i in range(ND)]
            wo_sbs = [const.tile([P, D], MM_DT, tag=f"wo{i}", name=f"wo{i}") for i in range(NEB)]
            bq_sb = const.tile([P, NEB], F32, tag="bq", name="bq")
            bk_sb = const.tile([P, NEB], F32, tag="bk", name="bk")
            # Q^T/K^T per (e-block, q-chunk); V per 512-wide k-chunk
            qts = [[const.tile([P, SC], MM_DT, tag=f"qt{e}{c}", name=f"qt{e}{c}") for c in range(NQ)]
                   for e in range(NEB)]
            kts = [[const.tile([P, SC], MM_DT, tag=f"kt{e}{c}", name=f"kt{e}{c}") for c in range(NQ)]
                   for e in range(NEB)]
            v_sbs = [const.tile([P, NQ, HPC * (DH + 1)], MM_DT, tag=f"v{i}", name=f"v{i}")
                     for i in range(NQ)]
            at_sbs = [[const.tile([P, SC], MM_DT, tag=f"at{i}{f}", name=f"at{i}{f}")
                       for f in range(NEB)] for i in range(NQ)]
            mk_sb = const.tile([P, NQ, SC], MM_DT, tag="mk", name="mk")
            # 33 partition rows of ones so slices at rows {0, 32} exist:
            # walrus requires matmul fmap+weight at the same partition base
            ones_sb = const.tile([DH // 2 + 1, DH], F32R, tag="ones", name="ones")
            # softmax-denominator staging: rows {0, 32} hold the two heads
            # of a pair (DVE partition bases must be 32-aligned); one
            # reciprocal instruction covers both. Per-e-block tiles so
            # adjacent chunks don't serialize on the buffer.
            d2_sbs = [const.tile([DH // 2 + 1, SC], F32R, tag=f"d2{e}", name=f"d2{e}")
                      for e in range(NEB)]
            rc2_sbs = [const.tile([DH // 2 + 1, SC], F32R, tag=f"rc2{e}", name=f"rc2{e}")
                       for e in range(NEB)]

            # input DMAs: k/q weights first (one coalesced transfer each),
            # then the first x column-half so chunk-0/1 projections can
            # start after ~3 MB instead of the full 5.5 MB
            nc.sync.dma_start(wk_sb[:], wkT.rearrange("(d p) e -> p d e", p=P))
            nc.sync.dma_start(wq_sb[:], wqT.rearrange("(d p) e -> p d e", p=P))
            for half in range(2):
                cols = slice(half * (S // 2), (half + 1) * (S // 2))
                for di in range(ND):
                    # alternate chunks across HWDGE/SWDGE queues for
                    # parallelism (SWDGE inside For_i fails codegen, so the
                    # benchmark repeat-loop build uses HWDGE only)
                    use_sw = (di % 2 == 1) and repeat == 1
                    dma = nc.gpsimd.dma_start if use_sw else nc.sync.dma_start
                    dma(x_sbs[di][:, cols], xT[di * P:(di + 1) * P, cols])
                if half == 0:
                    nc.sync.dma_start(wv_sb[:],
                                      wvT.rearrange("(d p) e -> p d e", p=P))
                    nc.sync.dma_start(bq_sb[:],
                                      bq.rearrange("(n p) -> p n", p=P))
                    nc.sync.dma_start(bk_sb[:],
                                      bk.rearrange("(n p) -> p n", p=P))
            for ft in range(NEB):
                nc.sync.dma_start(wo_sbs[ft][:], woT[ft * P:(ft + 1) * P, :])

            # constants: ones + multiplicative causal masks
            # (affine_select/memset can't write bf16; build f32, DVE-round)
            tmp = ctx.enter_context(tc.tile_pool(name="tmp", bufs=1))
            one_f32 = tmp.tile([P, 1], F32, tag="onef", name="onef")
            nc.vector.memset(one_f32[:], 1.0)
            nc.vector.tensor_copy(
                ones_sb[:],
                one_f32[0:DH // 2 + 1, 0:1].broadcast_to([DH // 2 + 1, DH]))
            # mk[m][kk, qq] = 1.0 if kk + 128*m <= qq else 0.0
            mkf_sb = tmp.tile([P, NQ, SC], F32, tag="mkf", name="mkf")
            for m in range(NQ):
                nc.gpsimd.memset(mkf_sb[:, m, :], 1.0)
                nc.gpsimd.affine_select(
                    out=mkf_sb[:, m, :], in_=mkf_sb[:, m, :],
                    compare_op=mybir.AluOpType.is_ge, fill=0.0,
                    base=-(P * m), pattern=[[1, SC]], channel_multiplier=-1,
                )
            nc.vector.tensor_copy(mk_sb[:], mkf_sb[:])
            for cc in range(NQ):
                nc.vector.tensor_copy(
                    v_sbs[cc][:, :, DH::DH + 1],
                    one_f32[:, :, None].broadcast_to([P, NQ, HPC]))
            for e in range(NEB):
                # fill 1.0 so the batched reciprocal's untouched lanes
                # (rows 1..31, 33+) read defined, non-zero values
                nc.vector.tensor_copy(
                    d2_sbs[e][:],
                    one_f32[0:DH // 2 + 1, 0:1].broadcast_to([DH // 2 + 1, SC]))

            # Dedicated PSUM pools so long-lived AV accumulators can't
            # starve score/projection/output tiles (8 banks total).
            pav = ctx.enter_context(tc.tile_pool(name="pav", bufs=pav_bufs, space="PSUM"))
            psc = ctx.enter_context(tc.tile_pool(name="psc", bufs=psc_bufs, space="PSUM"))
            pmx = ctx.enter_context(tc.tile_pool(name="pmx", bufs=pmx_bufs, space="PSUM"))
            ptp = ctx.enter_context(tc.tile_pool(name="ptp", bufs=ptp_bufs))
            rcp = ctx.enter_context(tc.tile_pool(name="rcp", bufs=2))
            obp = ctx.enter_context(tc.tile_pool(name="obp", bufs=3))

            # ---- PE filler machinery: the attention inner loop is ACT
            # (exp)-bound, so projection / output-projection matmul groups
            # are queued as work units and emitted into the attention
            # stream to fill the PE stall windows. fq_out has priority
            # (cheap units whose deps are already met); fq_proj units carry
            # their chunk id so attention never runs ahead of its own
            # q/k/v. All deps flow backward in emission order, so the Tile
            # semaphores stay acyclic.
            fq_out = []    # [(cost_ns, fn)]
            fq_proj = []   # [(chunk, cost_ns, fn)]
            budget = [0.0]
            # emissions postponed so a PE consumer never sits at the queue
            # head waiting on a long DVE chain (softmax reciprocal): flushed
            # a couple of attention blocks into the NEXT pair, by which time
            # the reciprocal has drained
            deferred = []

            def flush_deferred():
                for fn in deferred:
                    fn()
                deferred.clear()

            def pump(ns):
                budget[0] += ns
                while True:
                    if fq_out and budget[0] >= fq_out[0][0]:
                        cost, fn = fq_out.pop(0)
                        fn()
                        budget[0] -= cost
                    elif fq_proj and budget[0] >= fq_proj[0][1]:
                        _, cost, fn = fq_proj.pop(0)
                        fn()
                        budget[0] -= cost
                    else:
                        break

            def drain_proj(c):
                while fq_proj and fq_proj[0][0] <= c:
                    fq_proj.pop(0)[2]()

            def qk_group(w_sbs, b_sb, o_tiles, c, eb):
                ps = psc.tile([P, SC], F32, tag="sc", name="sc")
                for di in range(ND):
                    nc.tensor.matmul(
                        ps[:],
                        lhsT=w_sbs[di][:, eb * P:(eb + 1) * P],
                        rhs=x_sbs[di][:, c * SC:(c + 1) * SC],
                        start=(di == 0), stop=(di == ND - 1),
                    )
                nc.vector.tensor_scalar_add(
                    out=o_tiles[eb][c][:], in0=ps[:],
                    scalar1=b_sb[:, eb:eb + 1])

            def v_group(c, kk):
                kb = c * NQ + kk
                ps = psc.tile([P, SC], F32, tag="sc", name="sc")
                for di in range(ND):
                    nc.tensor.matmul(
                        ps[:, :E],
                        lhsT=x_sbs[di][:, kb * P:(kb + 1) * P],
                        rhs=wv_sbs[di][:],
                        start=(di == 0), stop=(di == ND - 1),
                    )
                dst = v_sbs[c][:, kk, :].rearrange(
                    "p (h e) -> p h e", h=HPC)[:, :, :DH]
                nc.vector.tensor_copy(
                    dst, ps[:, :E].rearrange("p (h e) -> p h e", h=HPC))

            def queue_proj(c):
                for eb in range(NEB):
                    fq_proj.append((c, 1730, lambda c=c, eb=eb: qk_group(
                        wk_sbs, bk_sb, kts, c, eb)))
                for eb in range(NEB):
                    fq_proj.append((c, 1730, lambda c=c, eb=eb: qk_group(
                        wq_sbs, bq_sb, qts, c, eb)))
                for kk in range(NQ):
                    fq_proj.append((c, 880, lambda c=c, kk=kk: v_group(c, kk)))

            def attention_pair(et, c):
                """Both heads of e-block et (partition rows 0:64 and 64:128 of
                the qt/kt tiles), interleaved per k-block so the two K=64
                score matmuls overlap in disjoint PE row-groups."""
                h0, h1 = 2 * et, 2 * et + 1
                avs = [pav.tile([DH + 1, SC], F32, tag="av", name="av")
                       for _ in range(2)]
                nkb_c = NQ * (c + 1)
                for j in range(nkb_c):
                    if j == 2:
                        flush_deferred()
                    m = j - NQ * c
                    # diagonal block at offset m: columns qq < 128m are fully
                    # masked -- restrict all work to [:, q0:]
                    q0 = P * m if m > 0 else 0
                    pss = []
                    for hi, er in ((0, 0), (1, DH)):
                        ps = psc.tile([P, SC], F32, tag="sc", name="sc")
                        nc.tensor.matmul(
                            ps[:, q0:],
                            lhsT=kts[et][j // NQ][er:er + DH,
                                                  (j % NQ) * P:(j % NQ + 1) * P],
                            rhs=qts[et][c][er:er + DH, q0:],
                            start=True, stop=True,
                        )
                        pss.append(ps)
                    pts = []
                    for hi in range(2):
                        pt = ptp.tile([P, SC], MM_DT, tag="pt", name="pt")
                        nc.scalar.activation(pt[:, q0:], pss[hi][:, q0:],
                                             AF.Exp, scale=SCALE)
                        if m >= 0:  # diagonal block: triangular mask
                            nc.vector.tensor_mul(pt[:, q0:], pt[:, q0:],
                                                 mk_sb[:, m, q0:])
                        pts.append(pt)
                    # fill the PE stall window while ACT runs the two exps
                    w = SC - q0
                    pump(max(0.0, 2 * (172 + w) / 1.2 - (3 * w / 2.4 + 80)))
                    for hi, h in ((0, h0), (1, h1)):
                        nc.tensor.matmul(
                            avs[hi][:, q0:],
                            lhsT=v_sbs[j // NQ][:, j % NQ,
                                                h * (DH + 1):(h + 1) * (DH + 1)],
                            rhs=pts[hi][:, q0:],
                            start=(j == 0), stop=(j == nkb_c - 1),
                        )
                # normalize: A^T[f, q] = av[f, q] * (1 / denom[q]); both
                # heads' denominators share one DVE reciprocal (2 lanes),
                # then each row is broadcast over 64 partitions via a K=1
                # outer product
                d2, rc2 = d2_sbs[et], rc2_sbs[et]
                with nc.allow_low_precision(
                        reason="f32r rounding of softmax recip is benign"):
                    for hi in range(2):
                        nc.vector.tensor_copy(d2[32 * hi:32 * hi + 1, :],
                                              avs[hi][DH:DH + 1, :])
                    nc.vector.reciprocal(rc2[:], d2[:])

                def finish_normalize(et=et, c=c, avs=avs, rc2=rc2):
                    for hi, er in ((0, 0), (1, DH)):
                        rb_ps = pmx.tile([DH, SC], F32, tag="b", name="psb")
                        nc.tensor.matmul(rb_ps[:],
                                         lhsT=ones_sb[32 * hi:32 * hi + 1, :],
                                         rhs=rc2[32 * hi:32 * hi + 1, :],
                                         start=True, stop=True,
                                         tile_position=(32 * hi, 0))
                        rcb = rcp.tile([DH, SC], MM_DT, tag="rcb", name="rcb")
                        nc.vector.tensor_copy(rcb[:], rb_ps[:])
                        with nc.allow_low_precision(
                                reason="bf16 attention output is in budget"):
                            nc.vector.tensor_mul(
                                at_sbs[c][et][er:er + DH, :],
                                avs[hi][0:DH, :], rcb[:])
                deferred.append(finish_normalize)

            def op_group(c, eb):
                po = pmx.tile([P, SC], F32, tag="b", name="psb")
                for ft in range(NEB):
                    nc.tensor.matmul(
                        po[:],
                        lhsT=wo_sbs[ft][:, eb * P:(eb + 1) * P],
                        rhs=at_sbs[c][ft][:],
                        start=(ft == 0), stop=(ft == NEB - 1),
                    )
                ob = obp.tile([P, SC], F32, tag="ob", name="ob")
                nc.vector.tensor_copy(ob[:], po[:])
                # SWDGE queue: output stores run parallel to input HWDGE
                dma = nc.gpsimd.dma_start if repeat == 1 else nc.sync.dma_start
                dma(outT[eb * P:(eb + 1) * P, c * SC:(c + 1) * SC], ob[:])

            # ---- schedule: chunk-0 projections up front (nothing to
            # overlap with yet), later chunks' projections + all output
            # projections flow into the attention loop's PE stall windows
            for eb in range(NEB):
                qk_group(wk_sbs, bk_sb, kts, 0, eb)
            for eb in range(NEB):
                qk_group(wq_sbs, bq_sb, qts, 0, eb)
            for kk in range(NQ):
                v_group(0, kk)
            for c in range(1, NQ):
                queue_proj(c)
            for c in range(NQ):
                drain_proj(c)     # q/k/v of chunk c must precede its scores
                for et in range(NEB):
                    attention_pair(et, c)

                def queue_out(c=c):
                    for eb in range(D // P):
                        fq_out.append(
                            (430, lambda c=c, eb=eb: op_group(c, eb)))
                # after finish_normalize(et=1, c) so out_proj matmuls never
                # precede their at-tile producers in the PE queue
                deferred.append(queue_out)
            flush_deferred()
            while fq_proj:
                fq_proj.pop(0)[2]()
            while fq_out:
                fq_out.pop(0)[1]()

    _split_multiwait(nc)
    return nc


_NC_CACHE = None
_last_in_maps = None


def kernel(**inputs):
    global _NC_CACHE, _last_in_maps
    if _NC_CACHE is None:
        _NC_CACHE = build_nc()
    nc = _NC_CACHE

    import ml_dtypes

    BF16 = ml_dtypes.bfloat16

    x = np.asarray(inputs["x"], np.float32)
    Wq = np.asarray(inputs["Wq"], np.float32)
    Wk = np.asarray(inputs["Wk"], np.float32)
    Wv = np.asarray(inputs["Wv"], np.float32)
    Wo = np.asarray(inputs["Wo"], np.float32)
    bq = np.asarray(inputs["bq"], np.float32)
    bk = np.asarray(inputs["bk"], np.float32)
    bv = np.asarray(inputs["bv"], np.float32)
    bo = np.asarray(inputs["bo"], np.float32)
    # The mask input is causal (tril ones) by construction; the kernel
    # hardcodes causal structure.

    xTs = [np.ascontiguousarray(x[b].T.astype(BF16)) for b in range(B)]
    in_maps = []
    for c in range(N_CORES):
        b, g = divmod(c, GROUPS)
        rows = slice(g * E, (g + 1) * E)
        in_maps.append({
            "xT": xTs[b],
            "wqT": np.ascontiguousarray(Wq[rows].T.astype(BF16)),
            "wkT": np.ascontiguousarray(Wk[rows].T.astype(BF16)),
            "wvT": np.ascontiguousarray(Wv[rows].T.astype(BF16)),
            "woT": np.ascontiguousarray(Wo[:, rows].T.astype(BF16)),
            "bq": np.ascontiguousarray(bq[rows]),
            "bk": np.ascontiguousarray(bk[rows]),
        })

    _last_in_maps = in_maps
    res = run_bass_kernel_spmd(nc, in_maps, list(range(N_CORES)))

    out = np.zeros((B, S, D), np.float32)
    for c in range(N_CORES):
        b = c // GROUPS
        out[b] += res.results[c]["outT"].T
    # bv enters only additively after softmax (rows of P sum to 1):
    # out += Wo @ bv; plus the output bias bo.
    out += (Wo @ bv + bo)[None, None, :]
    return out
